# revision 3
# baseline (speedup 1.0000x reference)
"""Trainium2 Bass kernel for CAttention:
    k      = einsum('bcit,i->bct', x, alpha)
    scores = einsum('bct,ts,bds->bcd', k, Wc, k)
    att    = softmax(scores, axis=-1)
    out    = einsum('bci,bint->bcnt', att, x)

Sharding: data-parallel over batch B=64 across 8 NeuronCores (8 batches/core).

Memory-bound: per-core traffic is x in (16.8MB fp16) + out (16.8MB fp16),
sharing ~358 GB/s HBM per core -> ~94us DMA floor.  v2 restructures the
baseline (158us) around that floor:

  * k-path in ONE fused custom-DVE op per X half: MULT_CUMSUM_ANT computes
    out[p,:] = cumsum(X * ac) in fp32 over a (t-major, n2-inner) strided AP,
    so per-t page sums fall out as differences of page-boundary cumsums.
    Replaces the multiply+7-op tree (12.2us DVE/batch -> ~7.5us) and is MORE
    accurate (fp32 accumulation of exact fp16 products vs fp16 tree level-1).
  * X rides ONE 2-half DMA per batch (8KB/partition contiguous descriptors
    vs 4KB quarters); out rides 2x 1MB half DMAs on the ACT ring.
  * PSUM evacuation in 1024-elem units (2 banks per unit, 2 matmuls each),
    split 6:2 between ACT and DVE in steady state (4:4 for the last batches).
  * Emission order keeps the out stream starting ~13us in (vs 48us): bd(b)
    is emitted before scan(b+1) on DVE; scans interleave between mix units.

Per-core layout (per batch b), as in v1:
    X SBUF tile fp16 [128, 8192]: partition p = j*8 + d  (j in [0,16) =
    n-chunk, d in [0,8) = channel), free q = n2*64 + t with n = j*128 + n2.
    smalls : kT = s @ sel (PE); V = Wc @ kT; scores = kT.T @ V; softmax on
             ACT (fp32, unnormalized exp + reciprocal rowsum on DVE)
    mix    : block-diag(att^T) fp16 [128,128] stationary, fp16 X moving
"""

import sys

for _p in ("/opt/trn_rl_repo",):
    if _p not in sys.path:
        sys.path.insert(0, _p)

import numpy as np

B, C, N, T = 64, 8, 2048, 64
NCORES = 8
BS = B // NCORES          # batches per core
J = 16                    # n-chunks on partitions
N2 = N // J               # 128, n-extent in free dim
P = J * C                 # 128 partitions
F = N2 * T                # 8192 free elems
FH = F // 2               # half (one in-DMA / one out-DMA unit)
N2H = N2 // 2             # 64 n2 values per half
UW = 1024                 # evac unit width (2 PSUM banks, 2 matmuls)

_PROGRAM_CACHE = {}

_CUSTOM_SHAS = {"v3": "738a75e9e385e48e", "v4": "f4b949e6ae385ae2"}


def _register_mult_cumsum():
    """Runtime-register the fused multiply+cumsum DVE op (no repo edits, so
    kernel.py stays self-contained).  out[p, k] = c0 + sum_{i<=k} in0*in1."""
    import concourse.dve_ops as dvo
    from concourse.dve_spec import AluOp, C0, Spec, Src0, Src1, scan

    for o in dvo.OPS:
        if o.name == "MULT_CUMSUM_ANT":
            return o

    def _ref(in0, in1, c0, c1, c2):
        p = in0.shape[0]
        prod = np.asarray(in0, np.float32).reshape(p, -1) * np.asarray(
            in1, np.float32
        ).reshape(p, -1)
        c0a = (
            c0.reshape(p, 1).astype(np.float32)
            if isinstance(c0, np.ndarray)
            else np.float32(c0)
        )
        return np.cumsum(prod, axis=1, dtype=np.float32) + c0a

    spec = Spec(body=scan(AluOp.ADD, Src0 * Src1, init=C0), reference=_ref)
    op = dvo.DveOp(
        "MULT_CUMSUM_ANT", spec, subdim=False, uops_sha=dict(_CUSTOM_SHAS)
    )
    used = set(dvo._SUB_OPCODE_FOR_NAME.values())
    try:
        from concourse.dve_table_gen import free_opcode_rows

        free = [r for r in free_opcode_rows("TRN2") if r not in used]
        row = free[0]
    except Exception:
        row = max(used) + 1
    assert row < 0x20
    dvo.OPS.append(op)
    dvo._SUB_OPCODE_FOR_NAME[op.name] = row
    dvo.CUSTOM_DVE_SPECS[op.name] = spec
    return op


def _build_program():
    from contextlib import ExitStack

    import concourse.bacc as bacc
    from concourse import mybir, tile

    cum_op = _register_mult_cumsum()

    fp32 = mybir.dt.float32
    fp16 = mybir.dt.float16
    nc = bacc.Bacc("TRN2", target_bir_lowering=False, debug=False)

    xs = nc.dram_tensor("xs", [BS, C, N, T], fp16, kind="ExternalInput").ap()
    # ac[(j*8+d), n2] = alpha[j*128+n2]  (fp16, 32KB)
    ac = nc.dram_tensor("ac", [P, N2], fp16, kind="ExternalInput").ap()
    # packed fp32: sel[0:8] | wcT[8:72] (rows 0-63) | id8[72:80] (rows 0-7) |
    #              rep[80:208] (rows 0-7) | mask[208:336]
    aux = nc.dram_tensor("aux", [P, 336], fp32, kind="ExternalInput").ap()
    out = nc.dram_tensor("out", [BS, C, N, T], fp16, kind="ExternalOutput").ap()

    Exp = mybir.ActivationFunctionType.Exp
    Copy = mybir.ActivationFunctionType.Copy
    ADD = mybir.AluOpType.add
    SUB = mybir.AluOpType.subtract

    with tile.TileContext(nc) as tc, ExitStack() as ctx:
        cpool = ctx.enter_context(tc.tile_pool(name="const", bufs=1))
        xpool = ctx.enter_context(tc.tile_pool(name="x", bufs=3))
        cumpool = ctx.enter_context(tc.tile_pool(name="cum", bufs=2))
        spool = ctx.enter_context(tc.tile_pool(name="small", bufs=2))
        bdpool = ctx.enter_context(tc.tile_pool(name="bd", bufs=2))
        opool = ctx.enter_context(tc.tile_pool(name="o", bufs=4))
        mixp = ctx.enter_context(tc.tile_pool(name="mixp", bufs=3, space="PSUM"))
        psmall = ctx.enter_context(tc.tile_pool(name="psmall", bufs=2, space="PSUM"))

        ac_t = cpool.tile([P, N2], fp16)
        aux_t = cpool.tile([P, 336], fp32)
        sel_t = aux_t[:, 0:8]
        wcT_t = aux_t[:T, 8:72]
        id8_t = aux_t[:C, 72:80]
        rep_t = aux_t[:C, 80:208]
        mask_t = aux_t[:, 208:336]

        def scan_part(X, n2lo, n2n, tag):
            """Fused X*ac cumsum over n2 range [n2lo, n2lo+n2n), pages = t.
            Returns cum tile [P, T, n2n] fp32; page-t sum over this n2 range
            is cum[:, t, -1] - cum[:, t-1, -1]."""
            cum = cumpool.tile([P, T, n2n], fp32, tag=tag)
            xv = X[:, n2lo * T : (n2lo + n2n) * T].rearrange(
                "p (n2 t) -> p t n2", t=T
            )
            av = (
                ac_t[:, n2lo : n2lo + n2n]
                .rearrange("p (x n2) -> p x n2", x=1)
                .to_broadcast([P, T, n2n])
            )
            nc.vector._custom_dve(cum_op, out=cum[:], in0=xv, in1=av, s0=0.0)
            return cum

        def lasts_ap(cum):
            # [P, T] strided view of the page-end cumsums
            n2n = cum.shape[2]
            return cum[:, :, n2n - 1 : n2n].rearrange("p t x -> p (t x)")

        def s_from_lastsum(b, lastsum):
            """s[p, t] = per-page sums: diff of the cumulative page ends."""
            s = spool.tile([P, T], fp32, tag="s")
            nc.vector.tensor_tensor(
                out=s[:, 1:T], in0=lastsum[:, 1:T], in1=lastsum[:, 0 : T - 1],
                op=SUB,
            )
            nc.vector.tensor_scalar_add(s[:, 0:1], lastsum[:, 0:1], 0.0)
            return s

        def phase_small1(b, s):
            """kT/V/scores/exp chain (PE + ACT only)."""
            kT_ps = psmall.tile([T, C], fp32, tag="ps")
            nc.tensor.matmul(kT_ps[:], lhsT=s[:, :T], rhs=sel_t, start=True, stop=True)
            kT_sb = spool.tile([T, C], fp32, tag="kTsb")
            nc.scalar.copy(kT_sb[:], kT_ps[:])

            v_ps = psmall.tile([T, C], fp32, tag="ps")
            nc.tensor.matmul(v_ps[:], lhsT=wcT_t, rhs=kT_sb[:], start=True, stop=True)
            v_sb = spool.tile([T, C], fp32, tag="vsb")
            nc.scalar.copy(v_sb[:], v_ps[:])

            sc_ps = psmall.tile([C, C], fp32, tag="ps")
            nc.tensor.matmul(sc_ps[:], lhsT=kT_sb[:], rhs=v_sb[:], start=True, stop=True)

            e_sb = spool.tile([C, C], fp32, tag="esb")
            ssum = spool.tile([C, 1], fp32, tag="ssum")
            nc.scalar.activation(e_sb[:], sc_ps[:], Exp, accum_out=ssum[:])
            return e_sb, ssum

        def emit_recip(ssum):
            rcp = spool.tile([C, 1], fp32, tag="rcp")
            nc.vector.reciprocal(rcp[:], ssum[:])
            return rcp

        def phase_small2(b, e_sb, rcp):
            """att normalize + att^T replication (PE + ACT only)."""
            att_sb = spool.tile([C, C], fp32, tag="attsb")
            nc.scalar.activation(att_sb[:], e_sb[:], Copy, scale=rcp[:])

            aT_ps = psmall.tile([C, C], fp32, tag="ps")
            nc.tensor.transpose(aT_ps[:], att_sb[:], id8_t)
            aT_sb = spool.tile([C, C], fp32, tag="aTsb")
            nc.scalar.copy(aT_sb[:], aT_ps[:])
            er_ps = psmall.tile([P, C], fp32, tag="ps")
            nc.tensor.matmul(er_ps[:], lhsT=rep_t, rhs=aT_sb[:], start=True, stop=True)
            return er_ps

        def emit_bd(b, er_ps):
            """bd[(j,d), (j',c)] = mask * erep  (block-diagonal att^T, fp16)."""
            bd = bdpool.tile([P, P], fp16, tag="bd")
            nc.vector.tensor_tensor(
                out=bd[:].rearrange("p (j c) -> p j c", j=J),
                in0=mask_t.rearrange("p (j c) -> p j c", j=J),
                in1=er_ps[:].rearrange("p (x c) -> p x c", x=1).to_broadcast([P, J, C]),
                op=mybir.AluOpType.mult,
            )
            return bd

        def mix_unit(b, X, bd, ost, u, on_dve):
            """One 1024-col mix unit: 2 matmuls into a 2-bank PSUM tile +
            one evacuation op on ACT or DVE."""
            mp = mixp.tile([P, UW], fp32, tag="mix")
            for h in range(2):
                lo = u * UW + h * 512
                nc.tensor.matmul(
                    mp[:, h * 512 : (h + 1) * 512],
                    lhsT=bd[:], rhs=X[:, lo : lo + 512],
                    start=True, stop=True,
                )
            osl = ost[:, (u % 4) * UW : (u % 4 + 1) * UW]
            if on_dve:
                nc.vector.tensor_scalar_add(osl, mp[:], 0.0)
            else:
                nc.scalar.activation(osl, mp[:], Copy)

        # ---------------- startup: batch 0 ----------------
        # X0 arrives in quarters (stream starts small); scans ride halves
        # (h0 waits only on quarters 0-1)
        nc.scalar.dma_start(ac_t[:], ac)
        X0 = xpool.tile([P, F], fp16, tag="X")
        xb0 = xs[0].rearrange("d (j n2) t -> j d (n2 t)", j=J)
        cums0 = []
        for q in range(4):
            sl = slice(q * (F // 4), (q + 1) * (F // 4))
            nc.sync.dma_start(X0[:, sl], xb0[:, :, sl])
            if q % 2 == 1:
                cums0.append(scan_part(X0, (q - 1) * (N2 // 4), N2H, tag="cumh"))
        nc.scalar.dma_start(aux_t[:], aux)

        # prefetch X1 halves
        X1 = xpool.tile([P, F], fp16, tag="X")
        xb1 = xs[1].rearrange("d (j n2) t -> j d (n2 t)", j=J)
        for h in range(2):
            sl = slice(h * FH, (h + 1) * FH)
            nc.sync.dma_start(X1[:, sl], xb1[:, :, sl])

        lastsum = spool.tile([P, T], fp32, tag="lastsum")
        nc.vector.tensor_tensor(
            out=lastsum[:], in0=lasts_ap(cums0[0]), in1=lasts_ap(cums0[1]), op=ADD
        )
        s = s_from_lastsum(0, lastsum)

        # ---------------- steady loop ----------------
        X = X0
        Xn = X1
        for b in range(BS):
            last = b == BS - 1
            e_sb, ssum = phase_small1(b, s)
            rcp = emit_recip(ssum)
            er_ps = phase_small2(b, e_sb, rcp)
            bd = emit_bd(b, er_ps)

            # prefetch X(b+2)
            if b + 2 < BS:
                X2 = xpool.tile([P, F], fp16, tag="X")
                xb2 = xs[b + 2].rearrange("d (j n2) t -> j d (n2 t)", j=J)
                for h in range(2):
                    sl = slice(h * FH, (h + 1) * FH)
                    nc.sync.dma_start(X2[:, sl], xb2[:, :, sl])
            else:
                X2 = None

            out_b = out[b].rearrange("c (j n2) t -> j c (n2 t)", j=J)
            # DVE evac units: 2 in steady state, 4 for the last two batches
            dve_units = (1, 3, 5, 7) if b >= BS - 2 else (3, 7)

            cum_h0 = cum_h1 = None
            ost = opool.tile([P, FH], fp16, tag="ost")
            for u in range(4):
                if u == 2 and not last:
                    cum_h0 = scan_part(Xn, 0, N2H, tag="cumh")
                mix_unit(b, X, bd, ost, u, on_dve=(u in dve_units))
            nc.scalar.dma_start(out_b[:, :, 0:FH], ost[:])

            ost = opool.tile([P, FH], fp16, tag="ost")
            for u in range(4, 8):
                if u == 6 and not last:
                    cum_h1 = scan_part(Xn, N2H, N2H, tag="cumh")
                mix_unit(b, X, bd, ost, u, on_dve=(u in dve_units))
            nc.scalar.dma_start(out_b[:, :, FH:F], ost[:])

            if not last:
                lastsum = spool.tile([P, T], fp32, tag="lastsum")
                nc.vector.tensor_tensor(
                    out=lastsum[:], in0=lasts_ap(cum_h0), in1=lasts_ap(cum_h1),
                    op=ADD,
                )
                s = s_from_lastsum(b + 1, lastsum)
                X = Xn
                Xn = X2

    nc.compile()
    return nc


def _host_constants(Wc: np.ndarray, alpha: np.ndarray):
    a = np.asarray(alpha, dtype=np.float16).reshape(J, N2)
    ac = np.repeat(a, C, axis=0)                         # [128, N2]
    sel = np.tile(np.eye(C, dtype=np.float32), (J, 1))
    id8 = np.eye(C, dtype=np.float32)
    rep = np.tile(np.eye(C, dtype=np.float32), (1, J))
    mask = np.kron(np.eye(J, dtype=np.float32), np.ones((C, C), dtype=np.float32))
    aux = np.zeros((P, 336), dtype=np.float32)
    aux[:, 0:8] = sel
    aux[:T, 8:72] = np.asarray(Wc.T, dtype=np.float32)
    aux[:C, 72:80] = id8
    aux[:C, 80:208] = rep
    aux[:, 208:336] = mask
    return {
        "ac": np.ascontiguousarray(ac),
        "aux": aux,
    }


def get_program():
    if "nc" not in _PROGRAM_CACHE:
        _PROGRAM_CACHE["nc"] = _build_program()
    return _PROGRAM_CACHE["nc"]


def run(x, Wc, alpha, trace=False, trace_kwargs=None):
    """Run on 8 cores; returns (full_output fp32, BassKernelResults)."""
    from concourse.bass_utils import run_bass_kernel_spmd

    nc = get_program()
    consts = _host_constants(np.asarray(Wc), np.asarray(alpha))
    x16 = np.asarray(x).astype(np.float16)
    in_maps = []
    for r in range(NCORES):
        m = {"xs": np.ascontiguousarray(x16[r * BS : (r + 1) * BS])}
        m.update(consts)
        in_maps.append(m)
    kw = {}
    if trace:
        kw["trace"] = True
        if trace_kwargs:
            kw.update(trace_kwargs)
    res = run_bass_kernel_spmd(nc, in_maps, list(range(NCORES)), **kw)
    out = np.concatenate([res.results[r]["out"] for r in range(NCORES)], axis=0)
    return out.astype(np.float32), res


def kernel(x, Wc, alpha):
    out, _ = run(x, Wc, alpha)
    return out


# revision 6
# speedup vs baseline: 1.2758x; 1.2758x over previous
"""Trainium2 Bass kernel for CAttention:
    k      = einsum('bcit,i->bct', x, alpha)
    scores = einsum('bct,ts,bds->bcd', k, Wc, k)
    att    = softmax(scores, axis=-1)
    out    = einsum('bci,bint->bcnt', att, x)

Sharding: data-parallel over batch B=64 across 8 NeuronCores (8 batches/core).

Memory-bound: per-core traffic is x in (16.8MB fp16) + out (16.8MB fp16).
Measured DMA model: each HWDGE ring (SP-issued, ACT-issued) sustains
~190-240 GB/s alone; both together reach the ~358 GB/s per-core HBM cap.
=> floor ~94us when in (SP ring) and out (ACT ring) overlap; startup and
drain alternate rings so single-stream phases also run at ~358.

v3 vs the 158us v1 baseline:
  * bd(b) is computed ONE iteration before mix(b): the smalls chain
    (kT->V->scores->exp->recip->att->aT->er, ~12 cross-engine hops) hides
    under the previous batch's 16 mix matmuls instead of gating its own.
  * tree keeps fp16 through L3 (2x DVE mode; L4+ fp32) - DVE k-path drops
    from 12.2 to ~10.7us/batch.  rel-err stays ~1e-2 < 2e-2 gate.
  * PSUM evacuation in 1024-elem units (2 banks, 2 matmuls each), all on
    ACT except one unit on DVE for odd batches (engine balance); the last
    two batches split 4/4 since DVE is idle there.
  * X arrives as 2x 1MB halves per batch (16KB/partition contiguous rows,
    quarters for batch 0 across both rings); out leaves as 2x 1MB halves
    (quarters alternating rings for the final batch's drain).

Per-core layout as v1: X fp16 [128, 8192], partition p = j*8+d, free
n2*64+t (n = j*128+n2); mix = block-diag(att^T) fp16 stationary.
"""

import sys

for _p in ("/opt/trn_rl_repo",):
    if _p not in sys.path:
        sys.path.insert(0, _p)

import numpy as np

B, C, N, T = 64, 8, 2048, 64
NCORES = 8
BS = B // NCORES          # batches per core
J = 16                    # n-chunks on partitions
N2 = N // J               # 128, n-extent in free dim
P = J * C                 # 128 partitions
F = N2 * T                # 8192 free elems
FH = F // 2
FQ = F // 4
UW = 1024                 # evac unit width (2 PSUM banks, 2 matmuls)

_PROGRAM_CACHE = {}


def _build_program():
    from contextlib import ExitStack

    import concourse.bacc as bacc
    from concourse import mybir, tile

    fp32 = mybir.dt.float32
    fp16 = mybir.dt.float16
    nc = bacc.Bacc("TRN2", target_bir_lowering=False, debug=False)

    xs = nc.dram_tensor("xs", [BS, C, N, T], fp16, kind="ExternalInput").ap()
    ac = nc.dram_tensor("ac", [P, N2], fp16, kind="ExternalInput").ap()
    # packed fp32: sel[0:8] | wcT[8:72] (rows 0-63) | id8[72:80] (rows 0-7) |
    #              rep[80:208] (rows 0-7) | mask[208:336]
    aux = nc.dram_tensor("aux", [P, 336], fp32, kind="ExternalInput").ap()
    out = nc.dram_tensor("out", [BS, C, N, T], fp16, kind="ExternalOutput").ap()

    Exp = mybir.ActivationFunctionType.Exp
    Copy = mybir.ActivationFunctionType.Copy
    ADD = mybir.AluOpType.add
    MULT = mybir.AluOpType.mult

    with tile.TileContext(nc) as tc, ExitStack() as ctx:
        cpool = ctx.enter_context(tc.tile_pool(name="const", bufs=1))
        xpool = ctx.enter_context(tc.tile_pool(name="x", bufs=3))
        apool = ctx.enter_context(tc.tile_pool(name="prod", bufs=2))
        scrpool = ctx.enter_context(tc.tile_pool(name="scr", bufs=2))
        opool = ctx.enter_context(tc.tile_pool(name="o", bufs=4))
        spool = ctx.enter_context(tc.tile_pool(name="small", bufs=2))
        bdpool = ctx.enter_context(tc.tile_pool(name="bd", bufs=3))
        mixp = ctx.enter_context(tc.tile_pool(name="mixp", bufs=3, space="PSUM"))
        psmall = ctx.enter_context(tc.tile_pool(name="psmall", bufs=2, space="PSUM"))

        ac_t = cpool.tile([P, N2], fp16)
        acf_t = cpool.tile([P, F], fp16)  # alpha replicated over t, on-device
        aux_t = cpool.tile([P, 336], fp32)
        sel_t = aux_t[:, 0:8]
        wcT_t = aux_t[:T, 8:72]
        id8_t = aux_t[:C, 72:80]
        rep_t = aux_t[:C, 80:208]
        mask_t = aux_t[:, 208:336]

        def xb_of(b):
            return xs[b].rearrange("d (j n2) t -> j d (n2 t)", j=J)

        def _acf_build_q(q):
            nc.vector.tensor_scalar_add(
                acf_t[:, q * FQ : (q + 1) * FQ].rearrange("p (n2 t) -> p n2 t", t=T),
                ac_t[:, q * (N2 // 4) : (q + 1) * (N2 // 4)]
                .rearrange("p (x n2) -> p n2 x", x=1)
                .to_broadcast([P, N2 // 4, T]),
                0.0,
            )

        def mult_q(prod, X, q):
            sl = slice(q * FQ, (q + 1) * FQ)
            nc.vector.tensor_tensor(
                out=prod[:, sl], in0=X[:, sl], in1=acf_t[:, sl], op=MULT
            )

        def tree(prod):
            """n2-reduction: fp16 in-place folds L1-L3 (2x DVE), fp32 L4+.
            Returns scr with s = scr[:, :T]."""
            w = F // 2
            while w >= 1024:                      # L1..L3 fp16, in-place
                nc.vector.tensor_tensor(
                    out=prod[:, :w], in0=prod[:, :w], in1=prod[:, w : 2 * w], op=ADD
                )
                w //= 2
            scr = scrpool.tile([P, 512], fp32, tag="scr")
            nc.vector.tensor_tensor(             # L4: fp16 pair -> fp32
                out=scr[:], in0=prod[:, :512], in1=prod[:, 512:1024], op=ADD
            )
            w = 256
            while w >= T:                        # L5..L7 fp32 in-place
                nc.vector.tensor_tensor(
                    out=scr[:, :w], in0=scr[:, :w], in1=scr[:, w : 2 * w], op=ADD
                )
                w //= 2
            return scr

        # ---- smalls chain pieces (emitted interleaved from the main loop) ----
        def small_kT(scr):
            kT_ps = psmall.tile([T, C], fp32, tag="ps")
            nc.tensor.matmul(kT_ps[:], lhsT=scr[:, :T], rhs=sel_t, start=True, stop=True)
            kT_sb = spool.tile([T, C], fp32, tag="kTsb")
            nc.scalar.copy(kT_sb[:], kT_ps[:])
            return kT_sb

        def small_V(kT_sb):
            v_ps = psmall.tile([T, C], fp32, tag="ps")
            nc.tensor.matmul(v_ps[:], lhsT=wcT_t, rhs=kT_sb[:], start=True, stop=True)
            v_sb = spool.tile([T, C], fp32, tag="vsb")
            nc.scalar.copy(v_sb[:], v_ps[:])
            return v_sb

        def small_scores_exp(kT_sb, v_sb):
            sc_ps = psmall.tile([C, C], fp32, tag="ps")
            nc.tensor.matmul(sc_ps[:], lhsT=kT_sb[:], rhs=v_sb[:], start=True, stop=True)
            e_sb = spool.tile([C, C], fp32, tag="esb")
            ssum = spool.tile([C, 1], fp32, tag="ssum")
            nc.scalar.activation(e_sb[:], sc_ps[:], Exp, accum_out=ssum[:])
            return e_sb, ssum

        def small_recip(ssum):
            rcp = spool.tile([C, 1], fp32, tag="rcp")
            nc.vector.reciprocal(rcp[:], ssum[:])
            return rcp

        def small_att_aT(e_sb, rcp):
            att_sb = spool.tile([C, C], fp32, tag="attsb")
            nc.scalar.activation(att_sb[:], e_sb[:], Copy, scale=rcp[:])
            aT_ps = psmall.tile([C, C], fp32, tag="ps")
            nc.tensor.transpose(aT_ps[:], att_sb[:], id8_t)
            aT_sb = spool.tile([C, C], fp32, tag="aTsb")
            nc.scalar.copy(aT_sb[:], aT_ps[:])
            return aT_sb

        def small_er(aT_sb):
            er_ps = psmall.tile([P, C], fp32, tag="ps")
            nc.tensor.matmul(er_ps[:], lhsT=rep_t, rhs=aT_sb[:], start=True, stop=True)
            return er_ps

        def emit_bd(er_ps):
            bd = bdpool.tile([P, P], fp16, tag="bd")
            nc.vector.tensor_tensor(
                out=bd[:].rearrange("p (j c) -> p j c", j=J),
                in0=mask_t.rearrange("p (j c) -> p j c", j=J),
                in1=er_ps[:].rearrange("p (x c) -> p x c", x=1).to_broadcast([P, J, C]),
                op=MULT,
            )
            return bd

        def mix_unit(X, bd, ost, u, on_dve):
            mp = mixp.tile([P, UW], fp32, tag="mix")
            for h in range(2):
                lo = u * UW + h * 512
                nc.tensor.matmul(
                    mp[:, h * 512 : (h + 1) * 512],
                    lhsT=bd[:], rhs=X[:, lo : lo + 512],
                    start=True, stop=True,
                )
            osl = ost[:, (u % 4) * UW : (u % 4 + 1) * UW]
            if on_dve:
                nc.vector.tensor_scalar_add(osl, mp[:], 0.0)
            else:
                nc.scalar.activation(osl, mp[:], Copy)

        # ================= startup =================
        # batch 0: X quarters alternate SP/ACT rings; acf built between
        nc.scalar.dma_start(ac_t[:], ac)
        X0 = xpool.tile([P, F], fp16, tag="X")
        xb0 = xb_of(0)
        for q in range(4):
            sl = slice(q * FQ, (q + 1) * FQ)
            if q % 2 == 0:
                nc.sync.dma_start(X0[:, sl], xb0[:, :, sl])
            else:
                nc.scalar.dma_start(X0[:, sl], xb0[:, :, sl])
            _acf_build_q(q)
        nc.scalar.dma_start(aux_t[:], aux)

        X1 = xpool.tile([P, F], fp16, tag="X")
        xb1 = xb_of(1)
        nc.sync.dma_start(X1[:, :FH], xb1[:, :, :FH])
        nc.scalar.dma_start(X1[:, FH:], xb1[:, :, FH:])

        prod0 = apool.tile([P, F], fp16, tag="prod")
        for q in range(4):
            mult_q(prod0, X0, q)
        scr0 = tree(prod0)

        # chain(0) + k-path(1); bd(0) emitted promptly after er(0)
        kT_sb = small_kT(scr0)
        prod1 = apool.tile([P, F], fp16, tag="prod")
        mult_q(prod1, X1, 0)
        v_sb = small_V(kT_sb)
        mult_q(prod1, X1, 1)
        e_sb, ssum = small_scores_exp(kT_sb, v_sb)
        rcp = small_recip(ssum)
        mult_q(prod1, X1, 2)
        aT_sb = small_att_aT(e_sb, rcp)
        er_ps = small_er(aT_sb)
        bd = emit_bd(er_ps)
        mult_q(prod1, X1, 3)
        scr = tree(prod1)

        X2 = xpool.tile([P, F], fp16, tag="X")
        xb2 = xb_of(2)
        nc.sync.dma_start(X2[:, :FH], xb2[:, :, :FH])
        nc.sync.dma_start(X2[:, FH:], xb2[:, :, FH:])

        # ================= steady loop =================
        # iteration b: mix(b) with bd(b) [ready], smalls(b+1) interleaved
        # between mix units, mult(b+2)+tree(b+2) on DVE, X(b+3) prefetch.
        X, Xn = X0, X1
        for b in range(BS):
            last = b == BS - 1
            do_next = not last          # smalls/bd for b+1
            do_next2 = b + 2 < BS       # k-path for b+2

            if do_next2:
                Xn2 = xpool.tile([P, F], fp16, tag="X")
                xbn = xb_of(b + 2)
                if b + 2 == BS - 1:
                    # last input: alternate rings to finish the in-stream fast
                    nc.sync.dma_start(Xn2[:, :FH], xbn[:, :, :FH])
                    nc.scalar.dma_start(Xn2[:, FH:], xbn[:, :, FH:])
                else:
                    nc.sync.dma_start(Xn2[:, :FH], xbn[:, :, :FH])
                    nc.sync.dma_start(Xn2[:, FH:], xbn[:, :, FH:])
                prodn = apool.tile([P, F], fp16, tag="prod")
            else:
                Xn2 = prodn = None

            out_b = out[b].rearrange("c (j n2) t -> j c (n2 t)", j=J)
            dve_units = (1, 3, 5, 7) if b >= BS - 2 else ((3,) if b % 2 else ())

            # ---- first half: mix u0-u3 + front of chain(b+1) ----
            ost = opool.tile([P, FH], fp16, tag="ost")
            if do_next:
                kT_sb = small_kT(scr)
            mix_unit(X, bd, ost, 0, on_dve=0 in dve_units)
            if do_next2:
                mult_q(prodn, Xn2, 0)
            if do_next:
                v_sb = small_V(kT_sb)
            mix_unit(X, bd, ost, 1, on_dve=1 in dve_units)
            if do_next:
                e_sb, ssum = small_scores_exp(kT_sb, v_sb)
                rcp = small_recip(ssum)
            mix_unit(X, bd, ost, 2, on_dve=2 in dve_units)
            if do_next2:
                mult_q(prodn, Xn2, 1)
            mix_unit(X, bd, ost, 3, on_dve=3 in dve_units)
            if last:
                # drain: quarters alternating rings
                nc.scalar.dma_start(out_b[:, :, 0:FQ], ost[:, :FQ])
                nc.sync.dma_start(out_b[:, :, FQ:FH], ost[:, FQ:])
            else:
                nc.scalar.dma_start(out_b[:, :, 0:FH], ost[:])

            # ---- second half: mix u4-u7 + back of chain(b+1) ----
            ost = opool.tile([P, FH], fp16, tag="ost")
            if do_next:
                aT_sb = small_att_aT(e_sb, rcp)
            mix_unit(X, bd, ost, 4, on_dve=4 in dve_units)
            if do_next2:
                mult_q(prodn, Xn2, 2)
            if do_next:
                er_ps = small_er(aT_sb)
            mix_unit(X, bd, ost, 5, on_dve=5 in dve_units)
            if do_next:
                bd_next = emit_bd(er_ps)
            mix_unit(X, bd, ost, 6, on_dve=6 in dve_units)
            if do_next2:
                mult_q(prodn, Xn2, 3)
            mix_unit(X, bd, ost, 7, on_dve=7 in dve_units)
            if last:
                nc.sync.dma_start(out_b[:, :, FH : FH + FQ], ost[:, :FQ])
                nc.scalar.dma_start(out_b[:, :, FH + FQ : F], ost[:, FQ:])
            else:
                nc.scalar.dma_start(out_b[:, :, FH:F], ost[:])

            if do_next2:
                scr = tree(prodn)
            if do_next:
                bd = bd_next
            X, Xn = Xn, Xn2

    nc.compile()
    return nc


def _host_constants(Wc: np.ndarray, alpha: np.ndarray):
    a = np.asarray(alpha, dtype=np.float16).reshape(J, N2)
    ac = np.repeat(a, C, axis=0)                         # [128, N2]
    sel = np.tile(np.eye(C, dtype=np.float32), (J, 1))
    id8 = np.eye(C, dtype=np.float32)
    rep = np.tile(np.eye(C, dtype=np.float32), (1, J))
    mask = np.kron(np.eye(J, dtype=np.float32), np.ones((C, C), dtype=np.float32))
    aux = np.zeros((P, 336), dtype=np.float32)
    aux[:, 0:8] = sel
    aux[:T, 8:72] = np.asarray(Wc.T, dtype=np.float32)
    aux[:C, 72:80] = id8
    aux[:C, 80:208] = rep
    aux[:, 208:336] = mask
    return {
        "ac": np.ascontiguousarray(ac),
        "aux": aux,
    }


def get_program():
    if "nc" not in _PROGRAM_CACHE:
        _PROGRAM_CACHE["nc"] = _build_program()
    return _PROGRAM_CACHE["nc"]


def run(x, Wc, alpha, trace=False, trace_kwargs=None):
    """Run on 8 cores; returns (full_output fp32, BassKernelResults)."""
    from concourse.bass_utils import run_bass_kernel_spmd

    nc = get_program()
    consts = _host_constants(np.asarray(Wc), np.asarray(alpha))
    x16 = np.asarray(x).astype(np.float16)
    in_maps = []
    for r in range(NCORES):
        m = {"xs": np.ascontiguousarray(x16[r * BS : (r + 1) * BS])}
        m.update(consts)
        in_maps.append(m)
    kw = {}
    if trace:
        kw["trace"] = True
        if trace_kwargs:
            kw.update(trace_kwargs)
    res = run_bass_kernel_spmd(nc, in_maps, list(range(NCORES)), **kw)
    out = np.concatenate([res.results[r]["out"] for r in range(NCORES)], axis=0)
    return out.astype(np.float32), res


def kernel(x, Wc, alpha):
    out, _ = run(x, Wc, alpha)
    return out


# revision 14
# speedup vs baseline: 1.4560x; 1.1412x over previous
"""Trainium2 Bass kernel for CAttention:
    k      = einsum('bcit,i->bct', x, alpha)
    scores = einsum('bct,ts,bds->bcd', k, Wc, k)
    att    = softmax(scores, axis=-1)
    out    = einsum('bci,bint->bcnt', att, x)

Sharding: data-parallel over batch B=64 across 8 NeuronCores (8 batches/core).

Memory-bound: per-core traffic is x in (16.8MB fp16) + out (16.8MB fp16).
Measured DMA model: each HWDGE ring (SP-issued, ACT-issued) sustains
~190-240 GB/s alone; both together reach the ~358 GB/s per-core HBM cap.
=> floor ~94us when in (SP ring) and out (ACT ring) overlap; startup and
drain alternate rings so single-stream phases also run at ~358.

v3 vs the 158us v1 baseline:
  * bd(b) is computed ONE iteration before mix(b): the smalls chain
    (kT->V->scores->exp->recip->att->aT->er, ~12 cross-engine hops) hides
    under the previous batch's 16 mix matmuls instead of gating its own.
  * tree keeps fp16 through L3 (2x DVE mode; L4+ fp32) - DVE k-path drops
    from 12.2 to ~10.7us/batch.  rel-err stays ~1e-2 < 2e-2 gate.
  * PSUM evacuation in 1024-elem units (2 banks, 2 matmuls each), all on
    ACT except one unit on DVE for odd batches (engine balance); the last
    two batches split 4/4 since DVE is idle there.
  * X arrives as 2x 1MB halves per batch (16KB/partition contiguous rows,
    quarters for batch 0 across both rings); out leaves as 2x 1MB halves
    (quarters alternating rings for the final batch's drain).

Per-core layout as v1: X fp16 [128, 8192], partition p = j*8+d, free
n2*64+t (n = j*128+n2); mix = block-diag(att^T) fp16 stationary.
"""

import sys

for _p in ("/opt/trn_rl_repo",):
    if _p not in sys.path:
        sys.path.insert(0, _p)

import numpy as np

B, C, N, T = 64, 8, 2048, 64
NCORES = 8
BS = B // NCORES          # batches per core
J = 16                    # n-chunks on partitions
N2 = N // J               # 128, n-extent in free dim
P = J * C                 # 128 partitions
F = N2 * T                # 8192 free elems
FH = F // 2
FQ = F // 4
UW = 1024                 # evac unit width (2 PSUM banks, 2 matmuls)

_PROGRAM_CACHE = {}


def _build_program():
    from contextlib import ExitStack

    import concourse.bacc as bacc
    from concourse import mybir, tile

    fp32 = mybir.dt.float32
    fp16 = mybir.dt.float16
    nc = bacc.Bacc("TRN2", target_bir_lowering=False, debug=False)

    xs = nc.dram_tensor("xs", [BS, C, N, T], fp16, kind="ExternalInput").ap()
    ac = nc.dram_tensor("ac", [P, N2], fp16, kind="ExternalInput").ap()
    # packed fp32: sel[0:8] | wcT[8:72] (rows 0-63) | id8[72:80] (rows 0-7) |
    #              rep[80:208] (rows 0-7) | mask[208:336]
    aux = nc.dram_tensor("aux", [P, 336], fp32, kind="ExternalInput").ap()
    out = nc.dram_tensor("out", [BS, C, N, T], fp16, kind="ExternalOutput").ap()

    Exp = mybir.ActivationFunctionType.Exp
    Copy = mybir.ActivationFunctionType.Copy
    ADD = mybir.AluOpType.add
    MULT = mybir.AluOpType.mult

    with tile.TileContext(nc) as tc, ExitStack() as ctx:
        cpool = ctx.enter_context(tc.tile_pool(name="const", bufs=1))
        xpool = ctx.enter_context(tc.tile_pool(name="x", bufs=4))
        apool = ctx.enter_context(tc.tile_pool(name="prod", bufs=2))
        scrpool = ctx.enter_context(tc.tile_pool(name="scr", bufs=2))
        opool = ctx.enter_context(tc.tile_pool(name="o", bufs=4))
        spool = ctx.enter_context(tc.tile_pool(name="small", bufs=2))
        bdpool = ctx.enter_context(tc.tile_pool(name="bd", bufs=3))
        mixp = ctx.enter_context(tc.tile_pool(name="mixp", bufs=3, space="PSUM"))
        psmall = ctx.enter_context(tc.tile_pool(name="psmall", bufs=2, space="PSUM"))

        ac_t = cpool.tile([P, N2], fp16)
        acf_t = cpool.tile([P, F], fp16)  # alpha replicated over t, on-device
        aux_t = cpool.tile([P, 336], fp32)
        sel_t = aux_t[:, 0:8]
        wcT_t = aux_t[:T, 8:72]
        id8_t = aux_t[:C, 72:80]
        rep_t = aux_t[:C, 80:208]
        mask_t = aux_t[:, 208:336]

        def xb_of(b):
            return xs[b].rearrange("d (j n2) t -> j d (n2 t)", j=J)

        def _acf_build_q(q):
            nc.vector.tensor_scalar_add(
                acf_t[:, q * FQ : (q + 1) * FQ].rearrange("p (n2 t) -> p n2 t", t=T),
                ac_t[:, q * (N2 // 4) : (q + 1) * (N2 // 4)]
                .rearrange("p (x n2) -> p n2 x", x=1)
                .to_broadcast([P, N2 // 4, T]),
                0.0,
            )

        def mult_q(prod, X, q):
            sl = slice(q * FQ, (q + 1) * FQ)
            nc.vector.tensor_tensor(
                out=prod[:, sl], in0=X[:, sl], in1=acf_t[:, sl], op=MULT
            )

        def tree_a(prod):
            """L1..L3: fp16 in-place folds (2x DVE mode)."""
            w = F // 2
            while w >= 1024:
                nc.vector.tensor_tensor(
                    out=prod[:, :w], in0=prod[:, :w], in1=prod[:, w : 2 * w], op=ADD
                )
                w //= 2

        def tree_b(prod):
            """L4..L7: fp32.  Returns scr with s = scr[:, :T]."""
            scr = scrpool.tile([P, 512], fp32, tag="scr")
            nc.vector.tensor_tensor(
                out=scr[:], in0=prod[:, :512], in1=prod[:, 512:1024], op=ADD
            )
            w = 256
            while w >= T:
                nc.vector.tensor_tensor(
                    out=scr[:, :w], in0=scr[:, :w], in1=scr[:, w : 2 * w], op=ADD
                )
                w //= 2
            return scr

        def dve_copy(dst, src):
            nc.vector.tensor_scalar_add(dst, src, 0.0)

        # ---- smalls chain pieces (PE matmul / DVE copy split) ----
        def kT_mm(scr):
            kT_ps = psmall.tile([T, C], fp32, tag="ps")
            nc.tensor.matmul(kT_ps[:], lhsT=scr[:, :T], rhs=sel_t, start=True, stop=True)
            return kT_ps

        def kT_cp(kT_ps):
            kT_sb = spool.tile([T, C], fp32, tag="kTsb")
            dve_copy(kT_sb[:], kT_ps[:])
            return kT_sb

        def v_mm(kT_sb):
            v_ps = psmall.tile([T, C], fp32, tag="ps")
            nc.tensor.matmul(v_ps[:], lhsT=wcT_t, rhs=kT_sb[:], start=True, stop=True)
            return v_ps

        def v_cp(v_ps):
            v_sb = spool.tile([T, C], fp32, tag="vsb")
            dve_copy(v_sb[:], v_ps[:])
            return v_sb

        def sc_exp(kT_sb, v_sb):
            sc_ps = psmall.tile([C, C], fp32, tag="ps")
            nc.tensor.matmul(sc_ps[:], lhsT=kT_sb[:], rhs=v_sb[:], start=True, stop=True)
            e_sb = spool.tile([C, C], fp32, tag="esb")
            ssum = spool.tile([C, 1], fp32, tag="ssum")
            nc.scalar.activation(e_sb[:], sc_ps[:], Exp, accum_out=ssum[:])
            return e_sb, ssum

        def recip_of(ssum):
            rcp = spool.tile([C, 1], fp32, tag="rcp")
            nc.vector.reciprocal(rcp[:], ssum[:])
            return rcp

        def att_aT_mm(e_sb, rcp):
            att_sb = spool.tile([C, C], fp32, tag="attsb")
            nc.scalar.activation(att_sb[:], e_sb[:], Copy, scale=rcp[:])
            aT_ps = psmall.tile([C, C], fp32, tag="ps")
            nc.tensor.transpose(aT_ps[:], att_sb[:], id8_t)
            return aT_ps

        def aT_cp(aT_ps):
            aT_sb = spool.tile([C, C], fp32, tag="aTsb")
            dve_copy(aT_sb[:], aT_ps[:])
            return aT_sb

        def er_mm(aT_sb):
            er_ps = psmall.tile([P, C], fp32, tag="ps")
            nc.tensor.matmul(er_ps[:], lhsT=rep_t, rhs=aT_sb[:], start=True, stop=True)
            return er_ps

        def emit_bd(er_ps):
            bd = bdpool.tile([P, P], fp16, tag="bd")
            nc.vector.tensor_tensor(
                out=bd[:].rearrange("p (j c) -> p j c", j=J),
                in0=mask_t.rearrange("p (j c) -> p j c", j=J),
                in1=er_ps[:].rearrange("p (x c) -> p x c", x=1).to_broadcast([P, J, C]),
                op=MULT,
            )
            return bd

        def mix_unit(X, bd, ost, u, on_dve):
            mp = mixp.tile([P, UW], fp32, tag="mix")
            for h in range(2):
                lo = u * UW + h * 512
                nc.tensor.matmul(
                    mp[:, h * 512 : (h + 1) * 512],
                    lhsT=bd[:], rhs=X[:, lo : lo + 512],
                    start=True, stop=True,
                )
            osl = ost[:, (u % 4) * UW : (u % 4 + 1) * UW]
            if on_dve:
                nc.vector.tensor_scalar_add(osl, mp[:], 0.0)
            else:
                nc.scalar.activation(osl, mp[:], Copy)

        # ================= startup =================
        # batch 0: X quarters alternate SP/ACT rings; acf built between
        nc.scalar.dma_start(ac_t[:], ac)
        Xt = {}
        Xt[0] = xpool.tile([P, F], fp16, tag="X", name="Xv")
        xb0 = xb_of(0)
        for q in range(4):
            sl = slice(q * FQ, (q + 1) * FQ)
            if q % 2 == 0:
                nc.sync.dma_start(Xt[0][:, sl], xb0[:, :, sl])
            else:
                nc.scalar.dma_start(Xt[0][:, sl], xb0[:, :, sl])
            _acf_build_q(q)
        nc.scalar.dma_start(aux_t[:], aux)

        Xt[1] = xpool.tile([P, F], fp16, tag="X", name="Xv")
        xb1 = xb_of(1)
        nc.sync.dma_start(Xt[1][:, :FH], xb1[:, :, :FH])
        nc.scalar.dma_start(Xt[1][:, FH:], xb1[:, :, FH:])
        Xt[2] = xpool.tile([P, F], fp16, tag="X", name="Xv")
        xb2 = xb_of(2)
        nc.sync.dma_start(Xt[2][:, :FH], xb2[:, :, :FH])
        nc.sync.dma_start(Xt[2][:, FH:], xb2[:, :, FH:])

        # k-path(0), then chain(0) interleaved with k-path(1)
        prod0 = apool.tile([P, F], fp16, tag="prod")
        for q in range(4):
            mult_q(prod0, Xt[0], q)
        tree_a(prod0)
        scr0 = tree_b(prod0)

        kT_ps_ = kT_mm(scr0)
        prod1 = apool.tile([P, F], fp16, tag="prod")
        mult_q(prod1, Xt[1], 0)
        kT_sb = kT_cp(kT_ps_)
        v_ps_ = v_mm(kT_sb)
        mult_q(prod1, Xt[1], 1)
        v_sb = v_cp(v_ps_)
        e_sb, ssum = sc_exp(kT_sb, v_sb)
        mult_q(prod1, Xt[1], 2)
        rcp = recip_of(ssum)
        aT_ps_ = att_aT_mm(e_sb, rcp)
        mult_q(prod1, Xt[1], 3)
        aT_sb = aT_cp(aT_ps_)
        er_ps_ = er_mm(aT_sb)
        tree_a(prod1)
        bd = emit_bd(er_ps_)
        scr = tree_b(prod1)

        # ================= steady loop =================
        # iteration b: mix(b) [bd(b) ready], smalls(b+1) interleaved between
        # mix units, k-path(b+2) on DVE [X(b+2) already resident], X(b+3) in.
        for b in range(BS):
            last = b == BS - 1
            do_next = not last          # smalls/bd for b+1
            do_next2 = b + 2 < BS       # k-path for b+2

            if b + 3 < BS:
                Xt[b + 3] = xpool.tile([P, F], fp16, tag="X", name="Xv")
                xb3 = xb_of(b + 3)
                if b + 3 == BS - 1:
                    # last input: alternate rings to finish the in-stream fast
                    nc.sync.dma_start(Xt[b + 3][:, :FH], xb3[:, :, :FH])
                    nc.scalar.dma_start(Xt[b + 3][:, FH:], xb3[:, :, FH:])
                else:
                    nc.sync.dma_start(Xt[b + 3][:, :FH], xb3[:, :, :FH])
                    nc.sync.dma_start(Xt[b + 3][:, FH:], xb3[:, :, FH:])
            X = Xt[b]
            Xn2 = Xt[b + 2] if do_next2 else None
            if do_next2:
                prodn = apool.tile([P, F], fp16, tag="prod")

            out_b = out[b].rearrange("c (j n2) t -> j c (n2 t)", j=J)
            dve_units = (1, 3, 5, 7) if b >= BS - 2 else ()

            # ---- mix u0-u7 with chain(b+1) and k-path(b+2) interleaved ----
            ost = opool.tile([P, FH], fp16, tag="ost")
            if do_next:
                kT_ps_ = kT_mm(scr)
            mix_unit(X, bd, ost, 0, on_dve=0 in dve_units)
            if do_next2:
                mult_q(prodn, Xn2, 0)
            if do_next:
                kT_sb = kT_cp(kT_ps_)
                v_ps_ = v_mm(kT_sb)
            mix_unit(X, bd, ost, 1, on_dve=1 in dve_units)
            if do_next2:
                mult_q(prodn, Xn2, 1)
            if do_next:
                v_sb = v_cp(v_ps_)
                e_sb, ssum = sc_exp(kT_sb, v_sb)
            mix_unit(X, bd, ost, 2, on_dve=2 in dve_units)
            if do_next2:
                mult_q(prodn, Xn2, 2)
            if do_next:
                rcp = recip_of(ssum)
            mix_unit(X, bd, ost, 3, on_dve=3 in dve_units)
            if last:
                nc.scalar.dma_start(out_b[:, :, 0:FQ], ost[:, :FQ])
                nc.sync.dma_start(out_b[:, :, FQ:FH], ost[:, FQ:])
            else:
                nc.scalar.dma_start(out_b[:, :, 0:FH], ost[:])

            ost = opool.tile([P, FH], fp16, tag="ost")
            if do_next:
                aT_ps_ = att_aT_mm(e_sb, rcp)
            mix_unit(X, bd, ost, 4, on_dve=4 in dve_units)
            if do_next2:
                mult_q(prodn, Xn2, 3)
            if do_next:
                aT_sb = aT_cp(aT_ps_)
                er_ps_ = er_mm(aT_sb)
            mix_unit(X, bd, ost, 5, on_dve=5 in dve_units)
            if do_next2:
                tree_a(prodn)
            if do_next:
                bd_next = emit_bd(er_ps_)
            mix_unit(X, bd, ost, 6, on_dve=6 in dve_units)
            mix_unit(X, bd, ost, 7, on_dve=7 in dve_units)
            if do_next2:
                scr = tree_b(prodn)
            if last:
                nc.sync.dma_start(out_b[:, :, FH : FH + FQ], ost[:, :FQ])
                nc.scalar.dma_start(out_b[:, :, FH + FQ : F], ost[:, FQ:])
            else:
                nc.scalar.dma_start(out_b[:, :, FH:F], ost[:])

            if do_next:
                bd = bd_next

    nc.compile()
    return nc


def _host_constants(Wc: np.ndarray, alpha: np.ndarray):
    a = np.asarray(alpha, dtype=np.float16).reshape(J, N2)
    ac = np.repeat(a, C, axis=0)                         # [128, N2]
    sel = np.tile(np.eye(C, dtype=np.float32), (J, 1))
    id8 = np.eye(C, dtype=np.float32)
    rep = np.tile(np.eye(C, dtype=np.float32), (1, J))
    mask = np.kron(np.eye(J, dtype=np.float32), np.ones((C, C), dtype=np.float32))
    aux = np.zeros((P, 336), dtype=np.float32)
    aux[:, 0:8] = sel
    aux[:T, 8:72] = np.asarray(Wc.T, dtype=np.float32)
    aux[:C, 72:80] = id8
    aux[:C, 80:208] = rep
    aux[:, 208:336] = mask
    return {
        "ac": np.ascontiguousarray(ac),
        "aux": aux,
    }


def get_program():
    if "nc" not in _PROGRAM_CACHE:
        _PROGRAM_CACHE["nc"] = _build_program()
    return _PROGRAM_CACHE["nc"]


def run(x, Wc, alpha, trace=False, trace_kwargs=None):
    """Run on 8 cores; returns (full_output fp32, BassKernelResults)."""
    from concourse.bass_utils import run_bass_kernel_spmd

    nc = get_program()
    consts = _host_constants(np.asarray(Wc), np.asarray(alpha))
    x16 = np.asarray(x).astype(np.float16)
    in_maps = []
    for r in range(NCORES):
        m = {"xs": np.ascontiguousarray(x16[r * BS : (r + 1) * BS])}
        m.update(consts)
        in_maps.append(m)
    kw = {}
    if trace:
        kw["trace"] = True
        if trace_kwargs:
            kw.update(trace_kwargs)
    res = run_bass_kernel_spmd(nc, in_maps, list(range(NCORES)), **kw)
    out = np.concatenate([res.results[r]["out"] for r in range(NCORES)], axis=0)
    return out.astype(np.float32), res


def kernel(x, Wc, alpha):
    out, _ = run(x, Wc, alpha)
    return out


# revision 15
# speedup vs baseline: 1.6035x; 1.1013x over previous
"""Trainium2 Bass kernel for CAttention:
    k      = einsum('bcit,i->bct', x, alpha)
    scores = einsum('bct,ts,bds->bcd', k, Wc, k)
    att    = softmax(scores, axis=-1)
    out    = einsum('bci,bint->bcnt', att, x)

Sharding: data-parallel over batch B=64 across 8 NeuronCores (8 batches/core).

Memory-bound: per-core traffic is x in (16.8MB fp16) + out (16.8MB fp16).
Measured DMA model: each HWDGE ring (SP-issued, ACT-issued) sustains
~190-240 GB/s alone; both together reach the ~358 GB/s per-core HBM cap.
=> floor ~94us when in (SP ring) and out (ACT ring) overlap; startup and
drain alternate rings so single-stream phases also run at ~358.

v3 vs the 158us v1 baseline:
  * bd(b) is computed ONE iteration before mix(b): the smalls chain
    (kT->V->scores->exp->recip->att->aT->er, ~12 cross-engine hops) hides
    under the previous batch's 16 mix matmuls instead of gating its own.
  * tree keeps fp16 through L3 (2x DVE mode; L4+ fp32) - DVE k-path drops
    from 12.2 to ~10.7us/batch.  rel-err stays ~1e-2 < 2e-2 gate.
  * PSUM evacuation in 1024-elem units (2 banks, 2 matmuls each), all on
    ACT except one unit on DVE for odd batches (engine balance); the last
    two batches split 4/4 since DVE is idle there.
  * X arrives as 2x 1MB halves per batch (16KB/partition contiguous rows,
    quarters for batch 0 across both rings); out leaves as 2x 1MB halves
    (quarters alternating rings for the final batch's drain).

Per-core layout as v1: X fp16 [128, 8192], partition p = j*8+d, free
n2*64+t (n = j*128+n2); mix = block-diag(att^T) fp16 stationary.
"""

import sys

for _p in ("/opt/trn_rl_repo",):
    if _p not in sys.path:
        sys.path.insert(0, _p)

import numpy as np

B, C, N, T = 64, 8, 2048, 64
NCORES = 8
BS = B // NCORES          # batches per core
J = 16                    # n-chunks on partitions
N2 = N // J               # 128, n-extent in free dim
P = J * C                 # 128 partitions
F = N2 * T                # 8192 free elems
FH = F // 2
FQ = F // 4
UW = 1024                 # evac unit width (2 PSUM banks, 2 matmuls)

_PROGRAM_CACHE = {}


def _build_program():
    from contextlib import ExitStack

    import concourse.bacc as bacc
    from concourse import mybir, tile

    fp32 = mybir.dt.float32
    fp16 = mybir.dt.float16
    nc = bacc.Bacc("TRN2", target_bir_lowering=False, debug=False)

    xs = nc.dram_tensor("xs", [BS, C, N, T], fp16, kind="ExternalInput").ap()
    ac = nc.dram_tensor("ac", [P, N2], fp16, kind="ExternalInput").ap()
    # packed fp32: sel[0:8] | wcT[8:72] (rows 0-63) | id8[72:80] (rows 0-7) |
    #              rep[80:208] (rows 0-7) | mask[208:336]
    aux = nc.dram_tensor("aux", [P, 336], fp32, kind="ExternalInput").ap()
    out = nc.dram_tensor("out", [BS, C, N, T], fp16, kind="ExternalOutput").ap()

    Exp = mybir.ActivationFunctionType.Exp
    Copy = mybir.ActivationFunctionType.Copy
    ADD = mybir.AluOpType.add
    MULT = mybir.AluOpType.mult

    with tile.TileContext(nc) as tc, ExitStack() as ctx:
        cpool = ctx.enter_context(tc.tile_pool(name="const", bufs=1))
        xpool = ctx.enter_context(tc.tile_pool(name="x", bufs=5))
        apool = ctx.enter_context(tc.tile_pool(name="prod", bufs=2))
        scrpool = ctx.enter_context(tc.tile_pool(name="scr", bufs=2))
        opool = ctx.enter_context(tc.tile_pool(name="o", bufs=4))
        spool = ctx.enter_context(tc.tile_pool(name="small", bufs=2))
        bdpool = ctx.enter_context(tc.tile_pool(name="bd", bufs=3))
        mixp = ctx.enter_context(tc.tile_pool(name="mixp", bufs=3, space="PSUM"))
        psmall = ctx.enter_context(tc.tile_pool(name="psmall", bufs=2, space="PSUM"))

        ac_t = cpool.tile([P, N2], fp16)
        acf_t = cpool.tile([P, F], fp16)  # alpha replicated over t, on-device
        aux_t = cpool.tile([P, 336], fp32)
        sel_t = aux_t[:, 0:8]
        wcT_t = aux_t[:T, 8:72]
        id8_t = aux_t[:C, 72:80]
        rep_t = aux_t[:C, 80:208]
        mask_t = aux_t[:, 208:336]

        def xb_of(b):
            return xs[b].rearrange("d (j n2) t -> j d (n2 t)", j=J)

        def _acf_build_q(q):
            nc.vector.tensor_scalar_add(
                acf_t[:, q * FQ : (q + 1) * FQ].rearrange("p (n2 t) -> p n2 t", t=T),
                ac_t[:, q * (N2 // 4) : (q + 1) * (N2 // 4)]
                .rearrange("p (x n2) -> p n2 x", x=1)
                .to_broadcast([P, N2 // 4, T]),
                0.0,
            )

        def mult_q(prod, X, q):
            sl = slice(q * FQ, (q + 1) * FQ)
            nc.vector.tensor_tensor(
                out=prod[:, sl], in0=X[:, sl], in1=acf_t[:, sl], op=MULT
            )

        def tree_a(prod):
            """L1..L3: fp16 in-place folds (2x DVE mode)."""
            w = F // 2
            while w >= 1024:
                nc.vector.tensor_tensor(
                    out=prod[:, :w], in0=prod[:, :w], in1=prod[:, w : 2 * w], op=ADD
                )
                w //= 2

        def tree_b(prod):
            """L4..L7: fp32.  Returns scr with s = scr[:, :T]."""
            scr = scrpool.tile([P, 512], fp32, tag="scr")
            nc.vector.tensor_tensor(
                out=scr[:], in0=prod[:, :512], in1=prod[:, 512:1024], op=ADD
            )
            w = 256
            while w >= T:
                nc.vector.tensor_tensor(
                    out=scr[:, :w], in0=scr[:, :w], in1=scr[:, w : 2 * w], op=ADD
                )
                w //= 2
            return scr

        def dve_copy(dst, src):
            nc.vector.tensor_scalar_add(dst, src, 0.0)

        # ---- smalls chain pieces (PE matmul / DVE copy split) ----
        def kT_mm(scr):
            kT_ps = psmall.tile([T, C], fp32, tag="ps")
            nc.tensor.matmul(kT_ps[:], lhsT=scr[:, :T], rhs=sel_t, start=True, stop=True)
            return kT_ps

        def kT_cp(kT_ps):
            kT_sb = spool.tile([T, C], fp32, tag="kTsb")
            dve_copy(kT_sb[:], kT_ps[:])
            return kT_sb

        def v_mm(kT_sb):
            v_ps = psmall.tile([T, C], fp32, tag="ps")
            nc.tensor.matmul(v_ps[:], lhsT=wcT_t, rhs=kT_sb[:], start=True, stop=True)
            return v_ps

        def v_cp(v_ps):
            v_sb = spool.tile([T, C], fp32, tag="vsb")
            dve_copy(v_sb[:], v_ps[:])
            return v_sb

        def sc_exp(kT_sb, v_sb):
            sc_ps = psmall.tile([C, C], fp32, tag="ps")
            nc.tensor.matmul(sc_ps[:], lhsT=kT_sb[:], rhs=v_sb[:], start=True, stop=True)
            e_sb = spool.tile([C, C], fp32, tag="esb")
            ssum = spool.tile([C, 1], fp32, tag="ssum")
            nc.scalar.activation(e_sb[:], sc_ps[:], Exp, accum_out=ssum[:])
            return e_sb, ssum

        def recip_of(ssum):
            rcp = spool.tile([C, 1], fp32, tag="rcp")
            nc.vector.reciprocal(rcp[:], ssum[:])
            return rcp

        def att_aT_mm(e_sb, rcp):
            att_sb = spool.tile([C, C], fp32, tag="attsb")
            nc.scalar.activation(att_sb[:], e_sb[:], Copy, scale=rcp[:])
            aT_ps = psmall.tile([C, C], fp32, tag="ps")
            nc.tensor.transpose(aT_ps[:], att_sb[:], id8_t)
            return aT_ps

        def aT_cp(aT_ps):
            aT_sb = spool.tile([C, C], fp32, tag="aTsb")
            dve_copy(aT_sb[:], aT_ps[:])
            return aT_sb

        def er_mm(aT_sb):
            er_ps = psmall.tile([P, C], fp32, tag="ps")
            nc.tensor.matmul(er_ps[:], lhsT=rep_t, rhs=aT_sb[:], start=True, stop=True)
            return er_ps

        def emit_bd(er_ps):
            bd = bdpool.tile([P, P], fp16, tag="bd")
            nc.vector.tensor_tensor(
                out=bd[:].rearrange("p (j c) -> p j c", j=J),
                in0=mask_t.rearrange("p (j c) -> p j c", j=J),
                in1=er_ps[:].rearrange("p (x c) -> p x c", x=1).to_broadcast([P, J, C]),
                op=MULT,
            )
            return bd

        def mix_unit(X, bd, ost, u, on_dve):
            mp = mixp.tile([P, UW], fp32, tag="mix")
            for h in range(2):
                lo = u * UW + h * 512
                nc.tensor.matmul(
                    mp[:, h * 512 : (h + 1) * 512],
                    lhsT=bd[:], rhs=X[:, lo : lo + 512],
                    start=True, stop=True,
                )
            osl = ost[:, (u % 4) * UW : (u % 4 + 1) * UW]
            if on_dve:
                nc.vector.tensor_scalar_add(osl, mp[:], 0.0)
            else:
                nc.scalar.activation(osl, mp[:], Copy)

        # ================= startup =================
        # batch 0: X quarters alternate SP/ACT rings; acf built between
        nc.scalar.dma_start(ac_t[:], ac)
        Xt = {}
        Xt[0] = xpool.tile([P, F], fp16, tag="X", name="Xv")
        xb0 = xb_of(0)
        for q in range(4):
            sl = slice(q * FQ, (q + 1) * FQ)
            if q % 2 == 0:
                nc.sync.dma_start(Xt[0][:, sl], xb0[:, :, sl])
            else:
                nc.scalar.dma_start(Xt[0][:, sl], xb0[:, :, sl])
            _acf_build_q(q)
        nc.scalar.dma_start(aux_t[:], aux)

        Xt[1] = xpool.tile([P, F], fp16, tag="X", name="Xv")
        xb1 = xb_of(1)
        nc.sync.dma_start(Xt[1][:, :FH], xb1[:, :, :FH])
        nc.scalar.dma_start(Xt[1][:, FH:], xb1[:, :, FH:])
        Xt[2] = xpool.tile([P, F], fp16, tag="X", name="Xv")
        xb2 = xb_of(2)
        nc.sync.dma_start(Xt[2][:, :FH], xb2[:, :, :FH])
        nc.sync.dma_start(Xt[2][:, FH:], xb2[:, :, FH:])

        # k-path(0), then chain(0) interleaved with k-path(1)
        prod0 = apool.tile([P, F], fp16, tag="prod")
        for q in range(4):
            mult_q(prod0, Xt[0], q)
        tree_a(prod0)
        scr0 = tree_b(prod0)

        kT_ps_ = kT_mm(scr0)
        prod1 = apool.tile([P, F], fp16, tag="prod")
        mult_q(prod1, Xt[1], 0)
        kT_sb = kT_cp(kT_ps_)
        v_ps_ = v_mm(kT_sb)
        mult_q(prod1, Xt[1], 1)
        v_sb = v_cp(v_ps_)
        e_sb, ssum = sc_exp(kT_sb, v_sb)
        mult_q(prod1, Xt[1], 2)
        rcp = recip_of(ssum)
        aT_ps_ = att_aT_mm(e_sb, rcp)
        mult_q(prod1, Xt[1], 3)
        aT_sb = aT_cp(aT_ps_)
        er_ps_ = er_mm(aT_sb)
        tree_a(prod1)
        bd = emit_bd(er_ps_)
        scr = tree_b(prod1)

        # ================= steady loop =================
        # iteration b: mix(b) [bd(b) ready], smalls(b+1) interleaved between
        # mix units, k-path(b+2) on DVE [X(b+2) already resident], X(b+3) in.
        for b in range(BS):
            last = b == BS - 1
            do_next = not last          # smalls/bd for b+1
            do_next2 = b + 2 < BS       # k-path for b+2

            if b + 3 < BS:
                Xt[b + 3] = xpool.tile([P, F], fp16, tag="X", name="Xv")
                xb3 = xb_of(b + 3)
                if b + 3 == BS - 1:
                    # last input: alternate rings to finish the in-stream fast
                    nc.sync.dma_start(Xt[b + 3][:, :FH], xb3[:, :, :FH])
                    nc.scalar.dma_start(Xt[b + 3][:, FH:], xb3[:, :, FH:])
                else:
                    nc.sync.dma_start(Xt[b + 3][:, :FH], xb3[:, :, :FH])
                    nc.sync.dma_start(Xt[b + 3][:, FH:], xb3[:, :, FH:])
            X = Xt[b]
            Xn2 = Xt[b + 2] if do_next2 else None
            if do_next2:
                prodn = apool.tile([P, F], fp16, tag="prod")

            out_b = out[b].rearrange("c (j n2) t -> j c (n2 t)", j=J)
            dve_units = (1, 3, 5, 7) if b >= BS - 2 else ()

            # ---- mix u0-u7 with chain(b+1) and k-path(b+2) interleaved ----
            ost = opool.tile([P, FH], fp16, tag="ost")
            if do_next:
                kT_ps_ = kT_mm(scr)
            mix_unit(X, bd, ost, 0, on_dve=0 in dve_units)
            if do_next2:
                mult_q(prodn, Xn2, 0)
            if do_next:
                kT_sb = kT_cp(kT_ps_)
                v_ps_ = v_mm(kT_sb)
            mix_unit(X, bd, ost, 1, on_dve=1 in dve_units)
            if do_next2:
                mult_q(prodn, Xn2, 1)
            if do_next:
                v_sb = v_cp(v_ps_)
                e_sb, ssum = sc_exp(kT_sb, v_sb)
            mix_unit(X, bd, ost, 2, on_dve=2 in dve_units)
            if do_next2:
                mult_q(prodn, Xn2, 2)
            if do_next:
                rcp = recip_of(ssum)
            mix_unit(X, bd, ost, 3, on_dve=3 in dve_units)
            if last:
                nc.scalar.dma_start(out_b[:, :, 0:FQ], ost[:, :FQ])
                nc.sync.dma_start(out_b[:, :, FQ:FH], ost[:, FQ:])
            else:
                nc.scalar.dma_start(out_b[:, :, 0:FH], ost[:])

            ost = opool.tile([P, FH], fp16, tag="ost")
            if do_next:
                aT_ps_ = att_aT_mm(e_sb, rcp)
            mix_unit(X, bd, ost, 4, on_dve=4 in dve_units)
            if do_next2:
                mult_q(prodn, Xn2, 3)
            if do_next:
                aT_sb = aT_cp(aT_ps_)
                er_ps_ = er_mm(aT_sb)
            mix_unit(X, bd, ost, 5, on_dve=5 in dve_units)
            if do_next2:
                tree_a(prodn)
            if do_next:
                bd_next = emit_bd(er_ps_)
            mix_unit(X, bd, ost, 6, on_dve=6 in dve_units)
            mix_unit(X, bd, ost, 7, on_dve=7 in dve_units)
            if do_next2:
                scr = tree_b(prodn)
            if last:
                nc.sync.dma_start(out_b[:, :, FH : FH + FQ], ost[:, :FQ])
                nc.scalar.dma_start(out_b[:, :, FH + FQ : F], ost[:, FQ:])
            else:
                nc.scalar.dma_start(out_b[:, :, FH:F], ost[:])

            if do_next:
                bd = bd_next

    nc.compile()
    return nc


def _host_constants(Wc: np.ndarray, alpha: np.ndarray):
    a = np.asarray(alpha, dtype=np.float16).reshape(J, N2)
    ac = np.repeat(a, C, axis=0)                         # [128, N2]
    sel = np.tile(np.eye(C, dtype=np.float32), (J, 1))
    id8 = np.eye(C, dtype=np.float32)
    rep = np.tile(np.eye(C, dtype=np.float32), (1, J))
    mask = np.kron(np.eye(J, dtype=np.float32), np.ones((C, C), dtype=np.float32))
    aux = np.zeros((P, 336), dtype=np.float32)
    aux[:, 0:8] = sel
    aux[:T, 8:72] = np.asarray(Wc.T, dtype=np.float32)
    aux[:C, 72:80] = id8
    aux[:C, 80:208] = rep
    aux[:, 208:336] = mask
    return {
        "ac": np.ascontiguousarray(ac),
        "aux": aux,
    }


def get_program():
    if "nc" not in _PROGRAM_CACHE:
        _PROGRAM_CACHE["nc"] = _build_program()
    return _PROGRAM_CACHE["nc"]


def run(x, Wc, alpha, trace=False, trace_kwargs=None):
    """Run on 8 cores; returns (full_output fp32, BassKernelResults)."""
    from concourse.bass_utils import run_bass_kernel_spmd

    nc = get_program()
    consts = _host_constants(np.asarray(Wc), np.asarray(alpha))
    x16 = np.asarray(x).astype(np.float16)
    in_maps = []
    for r in range(NCORES):
        m = {"xs": np.ascontiguousarray(x16[r * BS : (r + 1) * BS])}
        m.update(consts)
        in_maps.append(m)
    kw = {}
    if trace:
        kw["trace"] = True
        if trace_kwargs:
            kw.update(trace_kwargs)
    res = run_bass_kernel_spmd(nc, in_maps, list(range(NCORES)), **kw)
    out = np.concatenate([res.results[r]["out"] for r in range(NCORES)], axis=0)
    return out.astype(np.float32), res


def kernel(x, Wc, alpha):
    out, _ = run(x, Wc, alpha)
    return out


# revision 16
# speedup vs baseline: 1.7389x; 1.0844x over previous
"""Trainium2 Bass kernel for CAttention:
    k      = einsum('bcit,i->bct', x, alpha)
    scores = einsum('bct,ts,bds->bcd', k, Wc, k)
    att    = softmax(scores, axis=-1)
    out    = einsum('bci,bint->bcnt', att, x)

Sharding: data-parallel over batch B=64 across 8 NeuronCores (8 batches/core).

Memory-bound: per-core traffic is x in (16.8MB fp16) + out (16.8MB fp16).
Measured DMA model: each HWDGE ring (SP-issued, ACT-issued) sustains
~190-240 GB/s alone; both together reach the ~358 GB/s per-core HBM cap.
=> floor ~94us when in (SP ring) and out (ACT ring) overlap; startup and
drain alternate rings so single-stream phases also run at ~358.

v3 vs the 158us v1 baseline:
  * bd(b) is computed ONE iteration before mix(b): the smalls chain
    (kT->V->scores->exp->recip->att->aT->er, ~12 cross-engine hops) hides
    under the previous batch's 16 mix matmuls instead of gating its own.
  * tree keeps fp16 through L3 (2x DVE mode; L4+ fp32) - DVE k-path drops
    from 12.2 to ~10.7us/batch.  rel-err stays ~1e-2 < 2e-2 gate.
  * PSUM evacuation in 1024-elem units (2 banks, 2 matmuls each), all on
    ACT except one unit on DVE for odd batches (engine balance); the last
    two batches split 4/4 since DVE is idle there.
  * X arrives as 2x 1MB halves per batch (16KB/partition contiguous rows,
    quarters for batch 0 across both rings); out leaves as 2x 1MB halves
    (quarters alternating rings for the final batch's drain).

Per-core layout as v1: X fp16 [128, 8192], partition p = j*8+d, free
n2*64+t (n = j*128+n2); mix = block-diag(att^T) fp16 stationary.
"""

import sys

for _p in ("/opt/trn_rl_repo",):
    if _p not in sys.path:
        sys.path.insert(0, _p)

import numpy as np

B, C, N, T = 64, 8, 2048, 64
NCORES = 8
BS = B // NCORES          # batches per core
J = 16                    # n-chunks on partitions
N2 = N // J               # 128, n-extent in free dim
P = J * C                 # 128 partitions
F = N2 * T                # 8192 free elems
FH = F // 2
FQ = F // 4
UW = 1024                 # evac unit width (2 PSUM banks, 2 matmuls)

_PROGRAM_CACHE = {}


def _build_program():
    from contextlib import ExitStack

    import concourse.bacc as bacc
    from concourse import mybir, tile

    fp32 = mybir.dt.float32
    fp16 = mybir.dt.float16
    nc = bacc.Bacc("TRN2", target_bir_lowering=False, debug=False)

    xs = nc.dram_tensor("xs", [BS, C, N, T], fp16, kind="ExternalInput").ap()
    ac = nc.dram_tensor("ac", [P, N2], fp16, kind="ExternalInput").ap()
    # packed fp32: sel[0:8] | wcT[8:72] (rows 0-63) | id8[72:80] (rows 0-7) |
    #              rep[80:208] (rows 0-7) | mask[208:336]
    aux = nc.dram_tensor("aux", [P, 336], fp32, kind="ExternalInput").ap()
    out = nc.dram_tensor("out", [BS, C, N, T], fp16, kind="ExternalOutput").ap()

    Exp = mybir.ActivationFunctionType.Exp
    Copy = mybir.ActivationFunctionType.Copy
    ADD = mybir.AluOpType.add
    MULT = mybir.AluOpType.mult

    with tile.TileContext(nc) as tc, ExitStack() as ctx:
        cpool = ctx.enter_context(tc.tile_pool(name="const", bufs=1))
        xpool = ctx.enter_context(tc.tile_pool(name="x", bufs=6))
        apool = ctx.enter_context(tc.tile_pool(name="prod", bufs=2))
        scrpool = ctx.enter_context(tc.tile_pool(name="scr", bufs=2))
        opool = ctx.enter_context(tc.tile_pool(name="o", bufs=4))
        spool = ctx.enter_context(tc.tile_pool(name="small", bufs=2))
        bdpool = ctx.enter_context(tc.tile_pool(name="bd", bufs=3))
        mixp = ctx.enter_context(tc.tile_pool(name="mixp", bufs=3, space="PSUM"))
        psmall = ctx.enter_context(tc.tile_pool(name="psmall", bufs=2, space="PSUM"))

        ac_t = cpool.tile([P, N2], fp16)
        acf_t = cpool.tile([P, F], fp16)  # alpha replicated over t, on-device
        aux_t = cpool.tile([P, 336], fp32)
        sel_t = aux_t[:, 0:8]
        wcT_t = aux_t[:T, 8:72]
        id8_t = aux_t[:C, 72:80]
        rep_t = aux_t[:C, 80:208]
        mask_t = aux_t[:, 208:336]

        def xb_of(b):
            return xs[b].rearrange("d (j n2) t -> j d (n2 t)", j=J)

        def _acf_build_q(q):
            nc.vector.tensor_scalar_add(
                acf_t[:, q * FQ : (q + 1) * FQ].rearrange("p (n2 t) -> p n2 t", t=T),
                ac_t[:, q * (N2 // 4) : (q + 1) * (N2 // 4)]
                .rearrange("p (x n2) -> p n2 x", x=1)
                .to_broadcast([P, N2 // 4, T]),
                0.0,
            )

        def mult_q(prod, X, q):
            sl = slice(q * FQ, (q + 1) * FQ)
            nc.vector.tensor_tensor(
                out=prod[:, sl], in0=X[:, sl], in1=acf_t[:, sl], op=MULT
            )

        def tree_a(prod):
            """L1..L3: fp16 in-place folds (2x DVE mode)."""
            w = F // 2
            while w >= 1024:
                nc.vector.tensor_tensor(
                    out=prod[:, :w], in0=prod[:, :w], in1=prod[:, w : 2 * w], op=ADD
                )
                w //= 2

        def tree_b(prod):
            """L4..L7: fp32.  Returns scr with s = scr[:, :T]."""
            scr = scrpool.tile([P, 512], fp32, tag="scr")
            nc.vector.tensor_tensor(
                out=scr[:], in0=prod[:, :512], in1=prod[:, 512:1024], op=ADD
            )
            w = 256
            while w >= T:
                nc.vector.tensor_tensor(
                    out=scr[:, :w], in0=scr[:, :w], in1=scr[:, w : 2 * w], op=ADD
                )
                w //= 2
            return scr

        def dve_copy(dst, src):
            nc.vector.tensor_scalar_add(dst, src, 0.0)

        # ---- smalls chain pieces (PE matmul / DVE copy split) ----
        def kT_mm(scr):
            kT_ps = psmall.tile([T, C], fp32, tag="ps")
            nc.tensor.matmul(kT_ps[:], lhsT=scr[:, :T], rhs=sel_t, start=True, stop=True)
            return kT_ps

        def kT_cp(kT_ps):
            kT_sb = spool.tile([T, C], fp32, tag="kTsb")
            dve_copy(kT_sb[:], kT_ps[:])
            return kT_sb

        def v_mm(kT_sb):
            v_ps = psmall.tile([T, C], fp32, tag="ps")
            nc.tensor.matmul(v_ps[:], lhsT=wcT_t, rhs=kT_sb[:], start=True, stop=True)
            return v_ps

        def v_cp(v_ps):
            v_sb = spool.tile([T, C], fp32, tag="vsb")
            dve_copy(v_sb[:], v_ps[:])
            return v_sb

        def sc_exp(kT_sb, v_sb):
            sc_ps = psmall.tile([C, C], fp32, tag="ps")
            nc.tensor.matmul(sc_ps[:], lhsT=kT_sb[:], rhs=v_sb[:], start=True, stop=True)
            e_sb = spool.tile([C, C], fp32, tag="esb")
            ssum = spool.tile([C, 1], fp32, tag="ssum")
            nc.scalar.activation(e_sb[:], sc_ps[:], Exp, accum_out=ssum[:])
            return e_sb, ssum

        def recip_of(ssum):
            rcp = spool.tile([C, 1], fp32, tag="rcp")
            nc.vector.reciprocal(rcp[:], ssum[:])
            return rcp

        def att_aT_mm(e_sb, rcp):
            att_sb = spool.tile([C, C], fp32, tag="attsb")
            nc.scalar.activation(att_sb[:], e_sb[:], Copy, scale=rcp[:])
            aT_ps = psmall.tile([C, C], fp32, tag="ps")
            nc.tensor.transpose(aT_ps[:], att_sb[:], id8_t)
            return aT_ps

        def aT_cp(aT_ps):
            aT_sb = spool.tile([C, C], fp32, tag="aTsb")
            dve_copy(aT_sb[:], aT_ps[:])
            return aT_sb

        def er_mm(aT_sb):
            er_ps = psmall.tile([P, C], fp32, tag="ps")
            nc.tensor.matmul(er_ps[:], lhsT=rep_t, rhs=aT_sb[:], start=True, stop=True)
            return er_ps

        def emit_bd(er_ps):
            bd = bdpool.tile([P, P], fp16, tag="bd")
            nc.vector.tensor_tensor(
                out=bd[:].rearrange("p (j c) -> p j c", j=J),
                in0=mask_t.rearrange("p (j c) -> p j c", j=J),
                in1=er_ps[:].rearrange("p (x c) -> p x c", x=1).to_broadcast([P, J, C]),
                op=MULT,
            )
            return bd

        def mix_unit(X, bd, ost, u, on_dve):
            mp = mixp.tile([P, UW], fp32, tag="mix")
            for h in range(2):
                lo = u * UW + h * 512
                nc.tensor.matmul(
                    mp[:, h * 512 : (h + 1) * 512],
                    lhsT=bd[:], rhs=X[:, lo : lo + 512],
                    start=True, stop=True,
                )
            osl = ost[:, (u % 4) * UW : (u % 4 + 1) * UW]
            if on_dve:
                nc.vector.tensor_scalar_add(osl, mp[:], 0.0)
            else:
                nc.scalar.activation(osl, mp[:], Copy)

        # ================= startup =================
        # batch 0: X quarters alternate SP/ACT rings; acf built between
        nc.scalar.dma_start(ac_t[:], ac)
        Xt = {}
        Xt[0] = xpool.tile([P, F], fp16, tag="X", name="Xv")
        xb0 = xb_of(0)
        for q in range(4):
            sl = slice(q * FQ, (q + 1) * FQ)
            if q % 2 == 0:
                nc.sync.dma_start(Xt[0][:, sl], xb0[:, :, sl])
            else:
                nc.scalar.dma_start(Xt[0][:, sl], xb0[:, :, sl])
            _acf_build_q(q)
        nc.scalar.dma_start(aux_t[:], aux)

        Xt[1] = xpool.tile([P, F], fp16, tag="X", name="Xv")
        xb1 = xb_of(1)
        nc.sync.dma_start(Xt[1][:, :FH], xb1[:, :, :FH])
        nc.scalar.dma_start(Xt[1][:, FH:], xb1[:, :, FH:])
        Xt[2] = xpool.tile([P, F], fp16, tag="X", name="Xv")
        xb2 = xb_of(2)
        nc.sync.dma_start(Xt[2][:, :FH], xb2[:, :, :FH])
        nc.sync.dma_start(Xt[2][:, FH:], xb2[:, :, FH:])
        Xt[3] = xpool.tile([P, F], fp16, tag="X", name="Xv")
        xb3s = xb_of(3)
        nc.sync.dma_start(Xt[3][:, :FH], xb3s[:, :, :FH])
        nc.sync.dma_start(Xt[3][:, FH:], xb3s[:, :, FH:])

        # k-path(0), then chain(0) interleaved with k-path(1)
        prod0 = apool.tile([P, F], fp16, tag="prod")
        for q in range(4):
            mult_q(prod0, Xt[0], q)
        tree_a(prod0)
        scr0 = tree_b(prod0)

        kT_ps_ = kT_mm(scr0)
        prod1 = apool.tile([P, F], fp16, tag="prod")
        mult_q(prod1, Xt[1], 0)
        kT_sb = kT_cp(kT_ps_)
        v_ps_ = v_mm(kT_sb)
        mult_q(prod1, Xt[1], 1)
        v_sb = v_cp(v_ps_)
        e_sb, ssum = sc_exp(kT_sb, v_sb)
        mult_q(prod1, Xt[1], 2)
        rcp = recip_of(ssum)
        aT_ps_ = att_aT_mm(e_sb, rcp)
        mult_q(prod1, Xt[1], 3)
        aT_sb = aT_cp(aT_ps_)
        er_ps_ = er_mm(aT_sb)
        tree_a(prod1)
        bd = emit_bd(er_ps_)
        scr = tree_b(prod1)

        # ================= steady loop =================
        # iteration b: mix(b) [bd(b) ready], smalls(b+1) interleaved between
        # mix units, k-path(b+2) on DVE [X(b+2) already resident], X(b+3) in.
        for b in range(BS):
            last = b == BS - 1
            do_next = not last          # smalls/bd for b+1
            do_next2 = b + 2 < BS       # k-path for b+2

            if b + 4 < BS:
                Xt[b + 4] = xpool.tile([P, F], fp16, tag="X", name="Xv")
                xb4 = xb_of(b + 4)
                nc.sync.dma_start(Xt[b + 4][:, :FH], xb4[:, :, :FH])
                nc.sync.dma_start(Xt[b + 4][:, FH:], xb4[:, :, FH:])
            X = Xt[b]
            Xn2 = Xt[b + 2] if do_next2 else None
            if do_next2:
                prodn = apool.tile([P, F], fp16, tag="prod")

            out_b = out[b].rearrange("c (j n2) t -> j c (n2 t)", j=J)
            dve_units = (1, 3, 5, 7) if b >= BS - 2 else ()

            # ---- mix u0-u7 with chain(b+1) and k-path(b+2) interleaved ----
            ost = opool.tile([P, FH], fp16, tag="ost")
            if do_next:
                kT_ps_ = kT_mm(scr)
            mix_unit(X, bd, ost, 0, on_dve=0 in dve_units)
            if do_next2:
                mult_q(prodn, Xn2, 0)
            if do_next:
                kT_sb = kT_cp(kT_ps_)
                v_ps_ = v_mm(kT_sb)
            mix_unit(X, bd, ost, 1, on_dve=1 in dve_units)
            if do_next2:
                mult_q(prodn, Xn2, 1)
            if do_next:
                v_sb = v_cp(v_ps_)
                e_sb, ssum = sc_exp(kT_sb, v_sb)
            mix_unit(X, bd, ost, 2, on_dve=2 in dve_units)
            if do_next2:
                mult_q(prodn, Xn2, 2)
            if do_next:
                rcp = recip_of(ssum)
            mix_unit(X, bd, ost, 3, on_dve=3 in dve_units)
            if last:
                nc.scalar.dma_start(out_b[:, :, 0:FQ], ost[:, :FQ])
                nc.sync.dma_start(out_b[:, :, FQ:FH], ost[:, FQ:])
            else:
                nc.scalar.dma_start(out_b[:, :, 0:FH], ost[:])

            ost = opool.tile([P, FH], fp16, tag="ost")
            if do_next:
                aT_ps_ = att_aT_mm(e_sb, rcp)
            mix_unit(X, bd, ost, 4, on_dve=4 in dve_units)
            if do_next2:
                mult_q(prodn, Xn2, 3)
            if do_next:
                aT_sb = aT_cp(aT_ps_)
                er_ps_ = er_mm(aT_sb)
            mix_unit(X, bd, ost, 5, on_dve=5 in dve_units)
            if do_next2:
                tree_a(prodn)
            if do_next:
                bd_next = emit_bd(er_ps_)
            mix_unit(X, bd, ost, 6, on_dve=6 in dve_units)
            mix_unit(X, bd, ost, 7, on_dve=7 in dve_units)
            if do_next2:
                scr = tree_b(prodn)
            if last:
                nc.sync.dma_start(out_b[:, :, FH : FH + FQ], ost[:, :FQ])
                nc.scalar.dma_start(out_b[:, :, FH + FQ : F], ost[:, FQ:])
            else:
                nc.scalar.dma_start(out_b[:, :, FH:F], ost[:])

            if do_next:
                bd = bd_next

    nc.compile()
    return nc


def _host_constants(Wc: np.ndarray, alpha: np.ndarray):
    a = np.asarray(alpha, dtype=np.float16).reshape(J, N2)
    ac = np.repeat(a, C, axis=0)                         # [128, N2]
    sel = np.tile(np.eye(C, dtype=np.float32), (J, 1))
    id8 = np.eye(C, dtype=np.float32)
    rep = np.tile(np.eye(C, dtype=np.float32), (1, J))
    mask = np.kron(np.eye(J, dtype=np.float32), np.ones((C, C), dtype=np.float32))
    aux = np.zeros((P, 336), dtype=np.float32)
    aux[:, 0:8] = sel
    aux[:T, 8:72] = np.asarray(Wc.T, dtype=np.float32)
    aux[:C, 72:80] = id8
    aux[:C, 80:208] = rep
    aux[:, 208:336] = mask
    return {
        "ac": np.ascontiguousarray(ac),
        "aux": aux,
    }


def get_program():
    if "nc" not in _PROGRAM_CACHE:
        _PROGRAM_CACHE["nc"] = _build_program()
    return _PROGRAM_CACHE["nc"]


def run(x, Wc, alpha, trace=False, trace_kwargs=None):
    """Run on 8 cores; returns (full_output fp32, BassKernelResults)."""
    from concourse.bass_utils import run_bass_kernel_spmd

    nc = get_program()
    consts = _host_constants(np.asarray(Wc), np.asarray(alpha))
    x16 = np.asarray(x).astype(np.float16)
    in_maps = []
    for r in range(NCORES):
        m = {"xs": np.ascontiguousarray(x16[r * BS : (r + 1) * BS])}
        m.update(consts)
        in_maps.append(m)
    kw = {}
    if trace:
        kw["trace"] = True
        if trace_kwargs:
            kw.update(trace_kwargs)
    res = run_bass_kernel_spmd(nc, in_maps, list(range(NCORES)), **kw)
    out = np.concatenate([res.results[r]["out"] for r in range(NCORES)], axis=0)
    return out.astype(np.float32), res


def kernel(x, Wc, alpha):
    out, _ = run(x, Wc, alpha)
    return out
